# revision 11
# baseline (speedup 1.0000x reference)
"""BlockSparseFFN (moe_routing) Trainium2 kernel — 8 NeuronCores, block-parallel
sparse SwiGLU.

Key observation: the STE mask is EXACTLY zero for non-top-16 blocks
((0-p)+p == 0 in IEEE f32) and 1±2e-7 for selected blocks, so only 16/64
blocks per token contribute — 4x less matmul work than the dense reference —
and the ~1 mask multiply can be skipped entirely (error ~2e-7 << 2e-2 gate).

Strategy:
- Host: fp64 router -> exact top-16 block ids per token (same top-k sets the
  f32 reference computes on this data). Instances (token, block) are grouped
  by block and packed into S "cells" per core: every core runs the same NEFF
  with per-cell capacities caps[s]; cell s on core c holds a contiguous
  piece of one block's token list (big blocks are split across cells, so
  padding is ~3%). Weights for each (cell, core) are gathered host-side into
  cell-major tensors; gathered x columns likewise (fp16).
- Device (per core, per cell): gate/up matmuls over the gathered tokens
  (fp16 weights/activations, f32 PSUM), SiLU*up -> hidden (fp16), 128-wide
  down-proj into instance-major partial outputs (fp16) streamed to HBM.
- Host: scatter-add the exactly-16 partials per token (reshape+sum).

fp16 keeps the 1 cycle/row PE rate of fp32r while halving DMA traffic and
has ~10x more precision than the 2e-2 gate needs.
"""
import sys

sys.path.insert(0, "/opt/trn_rl_repo")
import numpy as np

import concourse.bass as bass
import concourse.mybir as mybir
import concourse.tile as tile
from concourse import bacc
from concourse.bass_utils import run_bass_kernel_spmd

N_CORES = 8
B, S, D = 4, 2048, 2048
N = B * S                # 8192 tokens
I = 8192                 # intermediate
NB = 64                  # blocks
BS = 128                 # block size
TOP_K = 16
KT = D // 128            # 16 k-tiles for gate/up contraction
CHUNK = 512              # moving free dim per matmul group
DC = 512                 # down-proj output d-chunk
P_MIN = 128              # packing: smallest peel size

F32 = mybir.dt.float32
F16 = mybir.dt.float16


def build_nc(caps, repeat=1):
    caps = tuple(int(c) for c in caps)
    NS = len(caps)
    TOT = sum(caps)
    nc = bacc.Bacc("TRN2", target_bir_lowering=False, debug=False, num_devices=N_CORES)
    xgT_d = nc.dram_tensor("xgT", [D, TOT], F16, kind="ExternalInput")
    wg_d = nc.dram_tensor("wg", [D, NS * BS], F16, kind="ExternalInput")
    wu_d = nc.dram_tensor("wu", [D, NS * BS], F16, kind="ExternalInput")
    wd_d = nc.dram_tensor("wd", [NS * BS, D], F16, kind="ExternalInput")
    out_d = nc.dram_tensor("out", [TOT, D], F16, kind="ExternalOutput")

    with tile.TileContext(nc) as tc:
        with tc.tile_pool(name="wpool", bufs=2) as wpool, \
             tc.tile_pool(name="xpool", bufs=4) as xpool, \
             tc.tile_pool(name="spool", bufs=2) as spool, \
             tc.tile_pool(name="hpool", bufs=2) as hpool, \
             tc.tile_pool(name="opool", bufs=3) as opool, \
             tc.tile_pool(name="psgu", bufs=4, space="PSUM") as psgu, \
             tc.tile_pool(name="psdn", bufs=4, space="PSUM") as psdn:

          for _rep in range(repeat):
            def emit_down(hid, wds, cw, row0, key):
                """Down-projection for a finished chunk: hid [128(i), cw] fp16
                x wds [128(i), D] -> instance-major out rows [cw, D]."""
                for st0 in range(0, cw, 128):
                    m = min(128, cw - st0)
                    ost = opool.tile([128, D], F16, tag="ost")
                    for dc in range(D // DC):
                        pd = psdn.tile([128, DC], F32, tag="pd",
                                       name=f"pd{key}_{st0}_{dc}")
                        nc.tensor.matmul(pd[:m, :], hid[:, st0:st0 + m],
                                         wds[:, dc * DC:(dc + 1) * DC],
                                         start=True, stop=True)
                        # 3:1 Act:DVE split keeps DVE clear of the psum-release
                        # critical path (DVE's mul gates gate/up bank reuse)
                        if dc % 4 == 3:
                            nc.vector.tensor_copy(ost[:m, dc * DC:(dc + 1) * DC], pd[:m, :])
                        else:
                            nc.scalar.copy(ost[:m, dc * DC:(dc + 1) * DC], pd[:m, :])
                    nc.scalar.dma_start(out_d.ap()[row0 + st0: row0 + st0 + m, :],
                                        ost[:m, :])

            pend = None   # down-proj lags one chunk so PE never waits on Act/DVE
            off = 0
            for s in range(NS):
                cap = caps[s]
                # stream this cell's weights: 1.5 MB fp16
                wgs = wpool.tile([128, KT, BS], F16, tag="wg", name=f"wg{_rep}_{s}")
                wus = wpool.tile([128, KT, BS], F16, tag="wu", name=f"wu{_rep}_{s}")
                wds = wpool.tile([128, D], F16, tag="wd", name=f"wd{_rep}_{s}")
                nc.gpsimd.dma_start(
                    wgs[:], wg_d.ap()[:, s * BS:(s + 1) * BS]
                        .rearrange("(kt p) i -> p kt i", p=128))
                nc.gpsimd.dma_start(
                    wus[:], wu_d.ap()[:, s * BS:(s + 1) * BS]
                        .rearrange("(kt p) i -> p kt i", p=128))
                nc.gpsimd.dma_start(wds[:], wd_d.ap()[s * BS:(s + 1) * BS, :])

                for c0 in range(0, cap, CHUNK):
                    cw = min(CHUNK, cap - c0)
                    xg = xpool.tile([128, KT, cw], F16, tag="xg", name=f"xg{_rep}_{s}_{c0}")
                    nc.sync.dma_start(
                        xg[:],
                        xgT_d.ap()[:, off + c0: off + c0 + cw]
                            .rearrange("(kt p) c -> p kt c", p=128))

                    pg = psgu.tile([128, cw], F32, tag="ps", name=f"pg{_rep}_{s}_{c0}")
                    pu = psgu.tile([128, cw], F32, tag="ps", name=f"pu{_rep}_{s}_{c0}")
                    for kt in range(KT):
                        nc.tensor.matmul(pg[:], wgs[:, kt, :], xg[:, kt, :],
                                         start=(kt == 0), stop=(kt == KT - 1))
                    for kt in range(KT):
                        nc.tensor.matmul(pu[:], wus[:, kt, :], xg[:, kt, :],
                                         start=(kt == 0), stop=(kt == KT - 1))

                    sg = spool.tile([128, cw], F32, tag="sg")
                    nc.scalar.activation(sg[:], pg[:], mybir.ActivationFunctionType.Silu)
                    hid = hpool.tile([128, cw], F16, tag="hid",
                                     name=f"hid{_rep}_{s}_{c0}")
                    nc.vector.tensor_mul(hid[:], sg[:], pu[:])

                    if pend is not None:
                        emit_down(*pend)
                    pend = (hid, wds, cw, off + c0, f"{_rep}_{s}_{c0}")
                off += cap
            if pend is not None:
                emit_down(*pend)
                pend = None
    nc.compile()
    return nc


def _route(x_flat, router_w1, router_w2):
    """fp64 router; top-16 block ids per token (matches reference f32 top-k)."""
    x64 = x_flat.astype(np.float64)
    r1 = x64 @ router_w1.astype(np.float64).T
    sl = r1 / (1.0 + np.exp(-r1))
    lg = sl @ router_w2.astype(np.float64).T        # [N, NB]
    top = np.argpartition(lg, NB - TOP_K, axis=1)[:, NB - TOP_K:]
    return np.ascontiguousarray(top.astype(np.int64))


def _pack(counts):
    """Peel packing: groups of 8 cells (one per core). Each iteration peels a
    piece of size p = 8th-largest remaining demand from the top 8 demands
    (zero waste); remainder fragments re-enter the pool. Tail (<8 demands or
    small peels) falls back to sorted grouping. Returns [(cap, [(block, lo,
    size) x <=8])] — piece i of a group runs on core i."""
    demands = sorted([(int(c), b, 0) for b, c in enumerate(counts) if c > 0],
                     reverse=True)
    groups = []
    while len(demands) > 8 and demands[7][0] >= P_MIN:
        p = demands[7][0]
        pieces, frags = [], []
        for sz, b, lo in demands[:8]:
            take = min(sz, p)
            pieces.append((b, lo, take))
            if sz - take > 0:
                frags.append((sz - take, b, lo + take))
        groups.append((p, pieces))
        demands = sorted(frags + demands[8:], reverse=True)
    for s in range(0, len(demands), 8):
        grp = demands[s:s + 8]
        groups.append((grp[0][0], [(b, lo, sz) for sz, b, lo in grp]))
    return groups


def plan(top):
    """Returns caps [S], cell_block [S, N_CORES] (block id or -1),
    col_tok / inst_tok [N_CORES, TOT] (xgT gather index; -1 marks padding)."""
    tok = np.repeat(np.arange(N, dtype=np.int64), TOP_K)
    blk = top.ravel()
    order = np.argsort(blk, kind="stable")
    tok_sorted = tok[order]
    counts = np.bincount(blk, minlength=NB)
    starts = np.zeros(NB + 1, np.int64)
    starts[1:] = np.cumsum(counts)

    groups = _pack(counts)
    NS = len(groups)
    caps = [g[0] for g in groups]
    TOT = sum(caps)
    cell_block = np.full((NS, N_CORES), -1, np.int64)
    col_tok = np.zeros((N_CORES, TOT), np.int64)
    inst_tok = np.full((N_CORES, TOT), -1, np.int64)
    off = 0
    for s, (cap, pieces) in enumerate(groups):
        for c, (b, lo, sz) in enumerate(pieces):
            cell_block[s, c] = b
            t = tok_sorted[starts[b] + lo: starts[b] + lo + sz]
            col_tok[c, off:off + sz] = t
            inst_tok[c, off:off + sz] = t
        off += cap
    return caps, cell_block, col_tok, inst_tok


def prepare(x, gate_w, up_w, down_w, router_w1, router_w2):
    """Host prep shared by kernel() and test.py: returns (caps, in_maps, inst_tok)."""
    x_flat = np.ascontiguousarray(np.asarray(x, dtype=np.float32)).reshape(N, D)
    gate_w = np.asarray(gate_w, dtype=np.float32)
    up_w = np.asarray(up_w, dtype=np.float32)
    down_w = np.asarray(down_w, dtype=np.float32)
    top = _route(x_flat, np.asarray(router_w1, np.float32), np.asarray(router_w2, np.float32))
    caps, cell_block, col_tok, inst_tok = plan(top)
    NS = len(caps)

    xT16 = np.ascontiguousarray(x_flat.T).astype(np.float16)   # [D, N]
    g16 = gate_w.astype(np.float16)                            # [I, D]
    u16 = up_w.astype(np.float16)
    d16 = down_w.astype(np.float16)                            # [D, I]
    in_maps = []
    for c in range(N_CORES):
        wg = np.zeros((D, NS * BS), np.float16)
        wu = np.zeros((D, NS * BS), np.float16)
        wd = np.zeros((NS * BS, D), np.float16)
        for s in range(NS):
            b = cell_block[s, c]
            if b < 0:
                continue
            wg[:, s * BS:(s + 1) * BS] = g16[b * BS:(b + 1) * BS, :].T
            wu[:, s * BS:(s + 1) * BS] = u16[b * BS:(b + 1) * BS, :].T
            wd[s * BS:(s + 1) * BS, :] = d16[:, b * BS:(b + 1) * BS].T
        xgT = xT16[:, col_tok[c]]
        in_maps.append({
            "xgT": np.ascontiguousarray(xgT),
            "wg": wg, "wu": wu, "wd": wd,
        })
    return caps, in_maps, inst_tok


_CACHE = {}


def _get_nc(caps):
    key = tuple(caps)
    if key not in _CACHE:
        _CACHE[key] = build_nc(caps)
    return _CACHE[key]


def kernel(x, gate_w, up_w, down_w, router_w1, router_w2):
    caps, in_maps, inst_tok = prepare(x, gate_w, up_w, down_w, router_w1, router_w2)
    nc = _get_nc(caps)
    res = run_bass_kernel_spmd(nc, in_maps, list(range(N_CORES)))
    outs = np.concatenate([res.results[c]["out"] for c in range(N_CORES)], axis=0)
    outs = outs.astype(np.float32)                 # [N_CORES*TOT, D]
    it = inst_tok.ravel()
    valid = it >= 0
    vals = outs[valid]
    toks = it[valid]
    ordr = np.argsort(toks, kind="stable")
    out = vals[ordr].reshape(N, TOP_K, D).sum(axis=1)
    return out.reshape(B, S, D)
